# revision 1
# baseline (speedup 1.0000x reference)
"""Trainium2 Bass kernel for CafeEmbeddingBagCollection (moe_routing).

Reference op: for each of N=204800 flat tokens, route to one of two
embedding tables (hot table at |q| if query_result q < 0, else hash table
at q % HASH), then sum-pool the per-token rows into B=4096 bags given by
`offsets`, producing [B, 128] f32.

Strategy (data-parallel, tables replicated on all 8 cores):
  * Host does LAYOUT ONLY: concatenates [hot_W; hash_W; zero_row] into a
    single [1000002, 128] table, and rearranges each core's query_results
    into a bag-per-partition layout [128, CHUNKS * lmax] (sentinel-padded
    when bags are uneven) so that partition p of chunk k holds the tokens
    of bag (k*128 + p).
  * Device computes the hot/hash routing per chunk (q % HASH via an exact
    f32 reciprocal-multiply with +-1 correction), then gathers rows with
    one indirect DMA per token column (the DGE consumes one offset per
    partition per call).  Pooling rides the DMA itself: every column of a
    chunk lands on the same [128, D] accumulator with compute_op=add
    (column 0 writes), so no vector-engine reduction is needed at all and
    the sum-pool is finished the moment the last column lands.  Columns
    are issued round-robin across the four 128-bag chunks so each
    accumulator's write->accumulate chain never stalls the descriptor
    generator.
  * Host concatenates the 8 per-core [512, 128] outputs.
"""

import os
import sys

import numpy as np

sys.path.insert(0, "/opt/trn_rl_repo")

# Problem constants (hardcoded per harness contract).
B = 4096
L = 50
N = B * L
D = 128
HOT = 500000
HASH = 500000
NCORES = 8
BC = B // NCORES  # bags per core = 512
CHUNKS = BC // 128  # 128-bag chunks per core = 4

TROWS = HOT + 1 + HASH + 1  # 1000002: [hot | hash | zero row]
ZR = TROWS - 1  # index of the all-zero row (padding target)
PADVAL = 1 << 30  # sentinel query value for padded token slots

_CACHE: dict = {}


def _build_nc(lmax: int):
    """Build the SPMD Bass program for bags padded to lmax tokens."""
    import concourse.bacc as bacc
    import concourse.bass as bass
    import concourse.tile as tile
    from concourse import mybir

    M = CHUNKS * lmax

    nc = bacc.Bacc(
        "TRN2",
        target_bir_lowering=False,
        debug=False,
        num_devices=NCORES,
    )

    q_in = nc.dram_tensor("q", [128, M], mybir.dt.int32, kind="ExternalInput")
    table_in = nc.dram_tensor(
        "table", [TROWS, D], mybir.dt.float32, kind="ExternalInput"
    )
    out_dram = nc.dram_tensor(
        "out", [BC, D], mybir.dt.float32, kind="ExternalOutput"
    )

    f32 = mybir.dt.float32
    i32 = mybir.dt.int32
    Alu = mybir.AluOpType

    with tile.TileContext(nc) as tc:
        with (
            tc.tile_pool(name="route", bufs=2) as route,
            tc.tile_pool(name="accp", bufs=1) as accp,
        ):
            # routing math per chunk (baseline-proven sequence)
            def route_cols(c0, w):
                q = route.tile([128, w], i32, tag=f"q{w}")
                nc.sync.dma_start(out=q[:], in_=q_in[:, c0 : c0 + w])
                qf = route.tile([128, w], f32, tag=f"qf{w}")
                nc.vector.tensor_copy(qf[:], q[:])
                hot = route.tile([128, w], f32, tag=f"hot{w}")
                nc.vector.tensor_scalar(hot[:], qf[:], -1.0, float(HOT), op0=Alu.mult, op1=Alu.min)
                kf = route.tile([128, w], f32, tag=f"kf{w}")
                nc.vector.tensor_scalar_mul(kf[:], qf[:], 1.0 / HASH)
                ki = route.tile([128, w], i32, tag=f"ki{w}")
                nc.vector.tensor_copy(ki[:], kf[:])
                nc.vector.tensor_copy(kf[:], ki[:])
                r = route.tile([128, w], f32, tag=f"r{w}")
                nc.vector.tensor_scalar_mul(kf[:], kf[:], float(HASH))
                nc.vector.tensor_tensor(r[:], qf[:], kf[:], op=Alu.subtract)
                c1 = route.tile([128, w], f32, tag=f"c1{w}")
                nc.vector.tensor_scalar(c1[:], r[:], 0.0, float(HASH), op0=Alu.is_lt, op1=Alu.mult)
                nc.vector.tensor_tensor(r[:], r[:], c1[:], op=Alu.add)
                nc.vector.tensor_scalar(c1[:], r[:], float(HASH), float(HASH), op0=Alu.is_ge, op1=Alu.mult)
                nc.vector.tensor_tensor(r[:], r[:], c1[:], op=Alu.subtract)
                nc.vector.tensor_scalar_add(r[:], r[:], float(HOT + 1))
                idxf = route.tile([128, w], f32, tag=f"idxf{w}")
                mask = route.tile([128, w], i32, tag=f"mask{w}")
                nc.vector.tensor_scalar(mask[:], qf[:], 0.0, None, op0=Alu.is_lt)
                nc.vector.select(idxf[:], mask[:], hot[:], r[:])
                pmask = route.tile([128, w], i32, tag=f"pmask{w}")
                nc.vector.tensor_scalar(pmask[:], qf[:], float(PADVAL), None, op0=Alu.is_equal)
                zr = route.tile([128, w], f32, tag=f"zr{w}")
                nc.vector.memset(zr[:], float(ZR))
                nc.vector.copy_predicated(idxf[:], pmask[:], zr[:])
                idx_c = route.tile([128, w], i32, tag=f"idx_c{c0}", name="idx_c")
                nc.vector.tensor_copy(idx_c[:], idxf[:])
                return idx_c

            # routes[ch] = [(lo, hi, idx_tile), ...]; chunk 0's first column
            # gets a dedicated 1-wide route chain so the very first
            # descriptor generation starts as early as possible (fill).
            routes = [[] for _ in range(CHUNKS)]
            with tc.high_priority():
                routes[0].append((0, 1, route_cols(0, 1)))
            if lmax > 1:
                routes[0].append((1, lmax, route_cols(1, lmax - 1)))
            for ch in range(1, CHUNKS):
                routes[ch].append(
                    (0, lmax, route_cols(ch * lmax, lmax))
                )

            def idx_col(ch, j):
                for lo, hi, tile_ in routes[ch]:
                    if lo <= j < hi:
                        return tile_[:, j - lo : j - lo + 1]
                raise AssertionError((ch, j))

            accs = [
                accp.tile([128, D], f32, tag=f"acc{ch}", name=f"acc{ch}")
                for ch in range(CHUNKS)
            ]

            # Sum-pool on the DMA: column j of chunk ch gathers 128 rows
            # (one offset per partition) straight onto acc[ch] with
            # compute_op=add.  Round-robin over chunks gives each
            # accumulator chain ~4 descriptor slots (~4us) between its
            # consecutive columns, far more than the transfer+semaphore
            # latency, so the pool engine never stalls.
            for j in range(lmax):
                for ch in range(CHUNKS):
                    nc.gpsimd.indirect_dma_start(
                        out=accs[ch][:],
                        out_offset=None,
                        in_=table_in[:],
                        in_offset=bass.IndirectOffsetOnAxis(
                            ap=idx_col(ch, j),
                            axis=0,
                        ),
                        bounds_check=TROWS - 1,
                        oob_is_err=False,
                        compute_op=Alu.bypass if j == 0 else Alu.add,
                    )
                    if j == lmax - 1:
                        nc.sync.dma_start(
                            out=out_dram[ch * 128 : (ch + 1) * 128, :],
                            in_=accs[ch][:],
                        )

    nc.compile()
    return nc


def _arrange_tokens(query_results: np.ndarray, offsets: np.ndarray):
    """Bag-per-partition token layout. Returns (arranged [B, lmax] int32, lmax)."""
    starts = offsets.astype(np.int64)
    ends = np.empty_like(starts)
    ends[:-1] = starts[1:]
    ends[-1] = N
    lens = np.maximum(ends - starts, 0)
    lmax = int(lens.max()) if lens.size else 0
    uniform = bool((starts == np.arange(B, dtype=np.int64) * L).all())
    if uniform:
        return query_results.reshape(B, L).astype(np.int32), L
    arranged = np.full((B, lmax), PADVAL, dtype=np.int32)
    for b in range(B):
        s, e = starts[b], ends[b]
        if e > s:
            arranged[b, : e - s] = query_results[s:e]
    return arranged, lmax


def _build_table(hot_W: np.ndarray, hash_W: np.ndarray) -> np.ndarray:
    table = np.empty((TROWS, D), dtype=np.float32)
    table[: HOT + 1] = hot_W
    table[HOT + 1 : HOT + 1 + HASH] = hash_W
    table[ZR] = 0.0
    return table


def kernel(feature_ids, offsets, query_results, hot_W, hash_W):
    from concourse.bass_utils import run_bass_kernel_spmd

    query_results = np.asarray(query_results, dtype=np.int32)
    offsets = np.asarray(offsets, dtype=np.int32)
    hot_W = np.ascontiguousarray(np.asarray(hot_W, dtype=np.float32))
    hash_W = np.ascontiguousarray(np.asarray(hash_W, dtype=np.float32))

    table = _build_table(hot_W, hash_W)

    arranged, lmax = _arrange_tokens(query_results, offsets)
    lmax = max(lmax, 1)

    if lmax not in _CACHE:
        _CACHE[lmax] = _build_nc(lmax)
    nc = _CACHE[lmax]

    in_maps = []
    for c in range(NCORES):
        rows = arranged[c * BC : (c + 1) * BC]  # [512, lmax]
        q_arr = (
            rows.reshape(CHUNKS, 128, lmax)
            .transpose(1, 0, 2)
            .reshape(128, CHUNKS * lmax)
        )
        in_maps.append({"q": np.ascontiguousarray(q_arr), "table": table})

    r = run_bass_kernel_spmd(nc, in_maps, list(range(NCORES)))
    globals()["LAST_RESULTS"] = r  # exposes exec_time_ns/trace to test harness
    out = np.concatenate([r.results[c]["out"] for c in range(NCORES)], axis=0)
    return out.astype(np.float32)

